# revision 8
# baseline (speedup 1.0000x reference)
"""KroneckerMessage GNN message passing on 8 TRN2 NeuronCores.

v2 design (see kernel_v1_baseline.py for the previous version):
- Node phase: shard nodes 8-way; each core computes its slice of
  h = relu(LN(node_feat @ W_node + b_node)) and writes it bf16 into a
  pair-packed table (2 nodes per 256B row, each node padded to 64 bf16);
  AllGather so every core has the full table h_pad [NPAD/2, 128] bf16.
- Edge phase: shard edges by dst range. Host buckets each core's edges
  into 128-node dst windows, padding every window to K subtiles of 128
  edge slots. Per window: ONE gpsimd.dma_gather fetches h rows for all
  src and dst slots (2K*128 indices, idx = node//2 fits int16 because of
  pair packing); a DVE copy + copy_predicated picks the node's half by
  parity. Per 128-edge subtile: DVE builds kron [128,400] bf16 via
  broadcast APs; PE transposes to [100,4x128]; ACT evacuates PSUM; 4
  accumulating bf16 matmuls vs W_kron chunks -> z [128e,128o] f32 in a
  batched PSUM tile; DVE bn_stats/bn_aggr per subtile + per-batch
  sqrt/recip/negmul (batched over B subtiles to amortize small-op cost);
  final ACT relu((z-mu)*rstd) -> y bf16; scatter-add into the window via
  one-hot matmul accumulated in PSUM; flush each window once to DRAM.
"""
import math
import os

import numpy as np
import ml_dtypes

import concourse.bacc as bacc
import concourse.bass as bass
import concourse.tile as tile
from concourse import mybir
from concourse.bass_utils import run_bass_kernel_spmd
from concourse.masks import make_identity

N_CORES = 8
P = 128
D_NODE = 20  # node projection dim
KRON = 400  # 20*20
KCH = 100  # kron rows per matmul chunk
NCH = 4  # kron chunks
LN_EPS = 1e-5
ROW = 128  # bf16 elems per packed pair-row (256B)
HALF = 64  # bf16 elems per node slot inside a pair-row
EXT = 32  # extracted elems per edge slot (>= D_NODE)

# module-level knobs (test.py pokes these)
TRACE = False
TRACE_DIR = None
USE_SIM = False

_BUILD_CACHE = {}


# --------------------------------------------------------------------------
# host-side prep
# --------------------------------------------------------------------------
def _prep(node_feat, W_node, b_node, g_node, beta_node, W_kron, b_kron,
          g_kron, beta_kron, src, dst):
    N, GF = node_feat.shape
    OUT = W_kron.shape[1]
    E = src.shape[0]
    assert GF % P == 0 and OUT == P and W_kron.shape[0] == KRON

    nodes_pc = int(math.ceil(N / (N_CORES * P))) * P
    npad = nodes_pc * N_CORES
    wpc = nodes_pc // P
    assert npad // 2 <= 32767, "pair-packed table must fit int16 indices"

    src = np.asarray(src, np.int64)
    dst = np.asarray(dst, np.int64)
    win = (dst // P).astype(np.int64)  # global window id
    counts = np.bincount(win, minlength=wpc * N_CORES)
    K = max(1, int(math.ceil(counts.max() / P)))
    slots_pw = K * P
    e_pc = wpc * slots_pw

    order = np.argsort(win, kind="stable")
    swin = win[order]
    starts = np.concatenate([[0], np.cumsum(counts)])
    rank = np.arange(E, dtype=np.int64) - starts[swin]
    slot = (swin // wpc) * e_pc + (swin % wpc) * slots_pw + rank

    gsrc = np.zeros(N_CORES * e_pc, np.int16)
    gdst = np.zeros(N_CORES * e_pc, np.int16)
    par_src = np.zeros(N_CORES * e_pc, np.float32)
    par_dst = np.zeros(N_CORES * e_pc, np.float32)
    dloc = np.full(N_CORES * e_pc, -1.0, np.float32)
    gsrc[slot] = (src[order] // 2).astype(np.int16)
    gdst[slot] = (dst[order] // 2).astype(np.int16)
    par_src[slot] = (src[order] % 2).astype(np.float32)
    par_dst[slot] = (dst[order] % 2).astype(np.float32)
    dloc[slot] = (dst[order] % P).astype(np.float32)

    # gather index tensor per (core, window): [128, 2*slots_pw/16] int16,
    # position j (= s'*128 + p) wrapped to (partition j%16, col j//16),
    # replicated 8x across partition groups.
    idxc = 2 * slots_pw // 16
    gidx = np.zeros((N_CORES, wpc, 128, idxc), np.int16)
    par = np.zeros((N_CORES, wpc, 2 * slots_pw), np.float32)
    for c in range(N_CORES):
        for w in range(wpc):
            b0 = c * e_pc + w * slots_pw
            lst = np.concatenate([gsrc[b0:b0 + slots_pw], gdst[b0:b0 + slots_pw]])
            wrapped = lst.reshape(-1, 16).T  # [16, idxc]
            gidx[c, w] = np.tile(wrapped, (8, 1))
            par[c, w] = np.concatenate(
                [par_src[b0:b0 + slots_pw], par_dst[b0:b0 + slots_pw]])

    nf_pad = np.zeros((npad, GF), np.float32)
    nf_pad[:N] = np.asarray(node_feat, np.float32)

    # W_node -> [P, GF//P, D_NODE]
    wn = np.asarray(W_node, np.float32).reshape(GF // P, P, D_NODE)
    wn = np.ascontiguousarray(wn.transpose(1, 0, 2))

    # W_kron chunks: [KCH, NCH, OUT] bf16
    wk = np.asarray(W_kron, np.float32).reshape(NCH, KCH, OUT)
    wk = np.ascontiguousarray(wk.transpose(1, 0, 2))

    flags = dict(
        has_bn=bool(np.any(np.asarray(b_node) != 0)),
        has_gn=bool(np.any(np.asarray(g_node) != 1)),
        has_betan=bool(np.any(np.asarray(beta_node) != 0)),
        has_bk=bool(np.any(np.asarray(b_kron) != 0)),
        has_gk=bool(np.any(np.asarray(g_kron) != 1)),
        has_betak=bool(np.any(np.asarray(beta_kron) != 0)),
    )

    cfg = dict(N=N, GF=GF, OUT=OUT, E=E, nodes_pc=nodes_pc, npad=npad,
               wpc=wpc, K=K, e_pc=e_pc, **flags)

    bf = ml_dtypes.bfloat16
    in_maps = []
    for c in range(N_CORES):
        m = dict(
            nf=nf_pad[c * nodes_pc:(c + 1) * nodes_pc],
            wn=wn,
            wk=wk.astype(bf),
            gidx=gidx[c],
            par=par[c].reshape(-1).astype(np.uint8),
            dloc=dloc[c * e_pc:(c + 1) * e_pc],
            b_node=np.asarray(b_node, np.float32),
            g_node=np.asarray(g_node, np.float32),
            beta_node=np.asarray(beta_node, np.float32),
            bk=np.asarray(b_kron, np.float32).astype(bf),
            g_kron=np.asarray(g_kron, np.float32).astype(bf),
            beta_kron=np.asarray(beta_kron, np.float32).astype(bf),
            iota_bf=np.arange(P, dtype=np.float32).astype(bf),
        )
        in_maps.append(m)
    return cfg, in_maps


# --------------------------------------------------------------------------
# device program
# --------------------------------------------------------------------------
def _build(cfg):
    GF, OUT = cfg["GF"], cfg["OUT"]
    nodes_pc, npad, wpc, K, e_pc = (cfg["nodes_pc"], cfg["npad"], cfg["wpc"],
                                    cfg["K"], cfg["e_pc"])
    FCH = GF // P
    f32, bf16, i16 = mybir.dt.float32, mybir.dt.bfloat16, mybir.dt.int16
    slots_pw = K * P
    idxc = 2 * slots_pw // 16
    # LN batch size (PSUM budget: z tile is B*512B, <= 1.5 banks at B=6)
    B = min(6, K)
    NB = int(math.ceil(K / B))

    nc = bacc.Bacc(num_devices=N_CORES)
    nf = nc.dram_tensor("nf", [nodes_pc, GF], f32, kind="ExternalInput")
    wn = nc.dram_tensor("wn", [P, FCH, D_NODE], f32, kind="ExternalInput")
    wk = nc.dram_tensor("wk", [KCH, NCH, OUT], bf16, kind="ExternalInput")
    gidx = nc.dram_tensor("gidx", [wpc, 128, idxc], i16, kind="ExternalInput")
    par = nc.dram_tensor("par", [wpc * 2 * slots_pw], mybir.dt.uint8,
                         kind="ExternalInput")
    dloc = nc.dram_tensor("dloc", [e_pc], f32, kind="ExternalInput")
    b_node = nc.dram_tensor("b_node", [D_NODE], f32, kind="ExternalInput")
    g_node = nc.dram_tensor("g_node", [D_NODE], f32, kind="ExternalInput")
    beta_node = nc.dram_tensor("beta_node", [D_NODE], f32, kind="ExternalInput")
    bk = nc.dram_tensor("bk", [OUT], bf16, kind="ExternalInput")
    g_kron = nc.dram_tensor("g_kron", [OUT], bf16, kind="ExternalInput")
    beta_kron = nc.dram_tensor("beta_kron", [OUT], bf16, kind="ExternalInput")
    iota_bf = nc.dram_tensor("iota_bf", [P], bf16, kind="ExternalInput")

    out_part = nc.dram_tensor("out_part", [nodes_pc, OUT], f32,
                              kind="ExternalOutput")
    h_pad_part = nc.dram_tensor("h_pad_part", [nodes_pc // 2, ROW], bf16)
    h_pad_full = nc.dram_tensor("h_pad_full", [npad // 2, ROW], bf16,
                                addr_space="Shared")

    ntiles = nodes_pc // P

    # ---------------- phase 1: h = relu(LN(nf @ W_node + b)) --------------
    with tile.TileContext(nc) as tc:
        with (
            tc.tile_pool(name="hconst", bufs=1) as hconst,
            tc.tile_pool(name="hsb", bufs=3) as hsb,
            tc.tile_pool(name="hps", bufs=2, space="PSUM") as hps,
            tc.tile_pool(name="hsmall", bufs=4) as hsmall,
        ):
            ident_f32 = hconst.tile([P, P], f32)
            make_identity(nc, ident_f32[:])
            wn_sb = hconst.tile([P, FCH, D_NODE], f32)
            nc.gpsimd.dma_start(out=wn_sb[:], in_=wn[:])
            eps_t = hconst.tile([P, 1], f32)
            nc.vector.memset(eps_t[:], LN_EPS)
            if cfg["has_bn"]:
                bn_b = hconst.tile([P, D_NODE], f32)
                nc.gpsimd.dma_start(
                    out=bn_b[:],
                    in_=bass.AP(tensor=b_node, offset=0,
                                ap=[[0, P], [1, D_NODE]]))
            if cfg["has_gn"]:
                gn_b = hconst.tile([P, D_NODE], f32)
                nc.gpsimd.dma_start(
                    out=gn_b[:],
                    in_=bass.AP(tensor=g_node, offset=0,
                                ap=[[0, P], [1, D_NODE]]))
            if cfg["has_betan"]:
                betan_b = hconst.tile([P, D_NODE], f32)
                nc.gpsimd.dma_start(
                    out=betan_b[:],
                    in_=bass.AP(tensor=beta_node, offset=0,
                                ap=[[0, P], [1, D_NODE]]))

            h_stage = hconst.tile([P, ntiles, HALF], bf16)
            nc.vector.memset(h_stage[:], 0.0)

            for t in range(ntiles):
                nf_t = hsb.tile([P, GF], f32, tag="nf_t")
                nc.sync.dma_start(out=nf_t[:], in_=nf[t * P:(t + 1) * P, :])
                nfT_ps = hps.tile([P, FCH, P], f32, tag="nfT_ps")
                for c in range(FCH):
                    nc.tensor.transpose(out=nfT_ps[:, c, :],
                                        in_=nf_t[:, c * P:(c + 1) * P],
                                        identity=ident_f32[:])
                nfT = hsb.tile([P, FCH, P], f32, tag="nfT")
                nc.vector.tensor_copy(out=nfT[:], in_=nfT_ps[:])
                z_ps = hps.tile([P, D_NODE], f32, tag="z_ps")
                for c in range(FCH):
                    nc.tensor.matmul(out=z_ps[:], lhsT=nfT[:, c, :],
                                     rhs=wn_sb[:, c, :], start=(c == 0),
                                     stop=(c == FCH - 1))
                if cfg["has_bn"]:
                    z_sb = hsb.tile([P, D_NODE], f32, tag="z_sb")
                    nc.vector.tensor_add(out=z_sb[:], in0=z_ps[:], in1=bn_b[:])
                    z_in = z_sb
                else:
                    z_in = z_ps
                stats = hsmall.tile([P, 6], f32, tag="stats")
                nc.vector.bn_stats(out=stats[:], in_=z_in[:])
                mv = hsmall.tile([P, 2], f32, tag="mv")
                nc.vector.bn_aggr(out=mv[:], in_=stats[:])
                sd = hsmall.tile([P, 1], f32, tag="sd")
                nc.scalar.activation(out=sd[:], in_=mv[:, 1:2],
                                     func=mybir.ActivationFunctionType.Sqrt,
                                     bias=eps_t[:], scale=1.0)
                rstd = hsmall.tile([P, 1], f32, tag="rstd")
                nc.vector.reciprocal(out=rstd[:], in_=sd[:])
                nmr = hsmall.tile([P, 1], f32, tag="nmr")
                nc.vector.tensor_scalar(out=nmr[:], in0=mv[:, 0:1],
                                        scalar1=rstd[:, 0:1], scalar2=-1.0,
                                        op0=mybir.AluOpType.mult,
                                        op1=mybir.AluOpType.mult)
                simple = not (cfg["has_gn"] or cfg["has_betan"])
                func = (mybir.ActivationFunctionType.Relu if simple
                        else mybir.ActivationFunctionType.Identity)
                nc.scalar.activation(out=h_stage[:, t, 0:D_NODE], in_=z_in[:],
                                     func=func, bias=nmr[:],
                                     scale=rstd[:, 0:1])
                if not simple:
                    if cfg["has_gn"]:
                        nc.vector.tensor_mul(out=h_stage[:, t, 0:D_NODE],
                                             in0=h_stage[:, t, 0:D_NODE],
                                             in1=gn_b[:])
                    if cfg["has_betan"]:
                        nc.vector.tensor_add(out=h_stage[:, t, 0:D_NODE],
                                             in0=h_stage[:, t, 0:D_NODE],
                                             in1=betan_b[:])
                    nc.vector.tensor_scalar_max(out=h_stage[:, t, 0:D_NODE],
                                                in0=h_stage[:, t, 0:D_NODE],
                                                scalar1=0.0)
            # node n = t*128 + p lands in pair-row t*64 + p//2, half p%2
            nc.sync.dma_start(
                out=h_pad_part.rearrange("(t p2) (pm j) -> (p2 pm) t j",
                                         p2=P // 2, pm=2, j=HALF),
                in_=h_stage[:])

    # ---------------- collective: AllGather h ----------------------------
    with (
        nc.Block() as block,
        nc.semaphore("cc_sem") as cc_sem,
    ):
        @block.gpsimd
        def _(gpsimd):
            gpsimd.collective_compute(
                "AllGather",
                mybir.AluOpType.bypass,
                replica_groups=[list(range(N_CORES))],
                ins=[h_pad_part[:]],
                outs=[h_pad_full[:]],
            ).then_inc(cc_sem)
            gpsimd.wait_ge(cc_sem, 1)

    # ---------------- phase 2: edges --------------------------------------
    simple_k = not (cfg["has_gk"] or cfg["has_betak"])
    h_rows = bass.AP(tensor=h_pad_full, offset=0,
                     ap=[[ROW, npad // 2], [1, ROW]])
    with tile.TileContext(nc) as tc:
        with (
            tc.tile_pool(name="econst", bufs=1) as econst,
            tc.tile_pool(name="eg", bufs=2) as eg,
            tc.tile_pool(name="esb", bufs=3) as esb,
            tc.tile_pool(name="eps_t", bufs=2, space="PSUM") as epsT,
            tc.tile_pool(name="eps_z", bufs=2, space="PSUM") as epsZ,
            tc.tile_pool(name="eps_a", bufs=2, space="PSUM") as epsA,
            tc.tile_pool(name="esmall", bufs=4) as esmall,
        ):
            ident_bf = econst.tile([P, P], bf16)
            make_identity(nc, ident_bf[:])
            iota_t = econst.tile([P, P], bf16)
            nc.gpsimd.dma_start(
                out=iota_t[:], in_=bass.AP(tensor=iota_bf, offset=0,
                                           ap=[[0, P], [1, P]]))
            eps_t2 = econst.tile([P, 1], f32)
            nc.vector.memset(eps_t2[:], LN_EPS)
            wk_sb = econst.tile([KCH, NCH, OUT], bf16)
            nc.gpsimd.dma_start(out=wk_sb[:], in_=wk[:])
            if cfg["has_bk"]:
                ones_row = econst.tile([1, P], bf16)
                nc.vector.memset(ones_row[:], 1.0)
                bk_sb = econst.tile([1, OUT], bf16)
                nc.gpsimd.dma_start(out=bk_sb[:], in_=bk[None, :])
            if cfg["has_gk"]:
                gk_b = econst.tile([P, OUT], bf16)
                nc.gpsimd.dma_start(
                    out=gk_b[:], in_=bass.AP(tensor=g_kron, offset=0,
                                             ap=[[0, P], [1, OUT]]))
            if cfg["has_betak"]:
                betak_b = econst.tile([P, OUT], bf16)
                nc.gpsimd.dma_start(
                    out=betak_b[:],
                    in_=bass.AP(tensor=beta_kron, offset=0,
                                ap=[[0, P], [1, OUT]]))

            for w in range(wpc):
                base = w * slots_pw
                dloc_w = eg.tile([P, K], f32, tag="dloc_w")
                nc.sync.dma_start(
                    out=dloc_w[:],
                    in_=bass.AP(tensor=dloc, offset=base,
                                ap=[[1, P], [P, K]]))
                par_w = eg.tile([P, 2 * K], mybir.dt.uint8, tag="par_w")
                nc.sync.dma_start(
                    out=par_w[:],
                    in_=bass.AP(tensor=par, offset=2 * base,
                                ap=[[1, P], [P, 2 * K]]))
                gidx_w = eg.tile([128, idxc], i16, tag="gidx_w")
                nc.sync.dma_start(out=gidx_w[:], in_=gidx[w])

                graw = eg.tile([P, 2 * K, ROW], bf16, tag="graw")
                nc.gpsimd.dma_gather(
                    graw[:], h_rows, gidx_w[:], 2 * slots_pw, 2 * slots_pw,
                    ROW, elem_step=ROW, single_packet=False)

                # pick node half by parity: hsel[:, s', :] =
                #   graw[:, s', par?64+ : 0+][0:EXT]
                # inner dim padded to 34 so the out AP stays 3-D (the sim's
                # np.where needs out/mask/data ndim to line up)
                hsel = eg.tile([P, 2 * K, EXT + 2], bf16, tag="hsel")
                nc.vector.tensor_copy(out=hsel[:, :, 0:EXT],
                                      in_=graw[:, :, 0:EXT])
                nc.vector.copy_predicated(
                    out=hsel[:, :, 0:EXT],
                    mask=par_w[:, :, None].to_broadcast([P, 2 * K, EXT]),
                    data=graw[:, :, HALF:HALF + EXT])

                acc_ps = epsA.tile([P, OUT], f32, tag="acc")

                for b in range(NB):
                    nsub = min(B, K - b * B)
                    z_all = epsZ.tile([P, B, OUT], f32, tag="z_all")
                    mv_all = esmall.tile([P, B, 2], f32, tag="mv_all")
                    for i in range(nsub):
                        s = b * B + i
                        kron = esb.tile([P, D_NODE, D_NODE], bf16, tag="kron")
                        nc.vector.tensor_tensor(
                            out=kron[:],
                            in0=hsel[:, s, 0:D_NODE, None].to_broadcast(
                                [P, D_NODE, D_NODE]),
                            in1=hsel[:, K + s, None, 0:D_NODE].to_broadcast(
                                [P, D_NODE, D_NODE]),
                            op=mybir.AluOpType.mult)
                        kv = kron[:].rearrange("p a b -> p (a b)")
                        psT = epsT.tile([KCH, NCH, P], bf16, tag="psT")
                        for c in range(NCH):
                            nc.tensor.transpose(
                                out=psT[:, c, :],
                                in_=kv[:, c * KCH:(c + 1) * KCH],
                                identity=ident_bf[:])
                        kron_sb = esb.tile([KCH, NCH, P], bf16, tag="kron_sb")
                        nc.scalar.activation(
                            out=kron_sb[:], in_=psT[:],
                            func=mybir.ActivationFunctionType.Copy)
                        zi = z_all[:, i, :]
                        nmm = NCH + (1 if cfg["has_bk"] else 0)
                        for c in range(NCH):
                            nc.tensor.matmul(out=zi, lhsT=kron_sb[:, c, :],
                                             rhs=wk_sb[:, c, :],
                                             start=(c == 0),
                                             stop=(c == nmm - 1))
                        if cfg["has_bk"]:
                            nc.tensor.matmul(out=zi, lhsT=ones_row[:],
                                             rhs=bk_sb[:], start=False,
                                             stop=True, skip_group_check=True)
                        stats = esmall.tile([P, 6], f32, tag="stats")
                        nc.vector.bn_stats(out=stats[:], in_=zi)
                        nc.vector.bn_aggr(out=mv_all[:, i, :], in_=stats[:])

                    # batched LN scalars over the B subtiles
                    sd_b = esmall.tile([P, B], f32, tag="sd_b")
                    nc.scalar.activation(
                        out=sd_b[:, 0:nsub], in_=mv_all[:, 0:nsub, 1],
                        func=mybir.ActivationFunctionType.Sqrt,
                        bias=eps_t2[:], scale=1.0)
                    rstd_b = esmall.tile([P, B], f32, tag="rstd_b")
                    nc.vector.reciprocal(out=rstd_b[:, 0:nsub],
                                         in_=sd_b[:, 0:nsub])
                    nmr_b = esmall.tile([P, B], f32, tag="nmr_b")
                    nc.vector.scalar_tensor_tensor(
                        out=nmr_b[:, 0:nsub], in0=mv_all[:, 0:nsub, 0],
                        scalar=-1.0, in1=rstd_b[:, 0:nsub],
                        op0=mybir.AluOpType.mult, op1=mybir.AluOpType.mult)

                    for i in range(nsub):
                        s = b * B + i
                        y_sb = esb.tile([P, OUT], bf16, tag="y")
                        func = (mybir.ActivationFunctionType.Relu if simple_k
                                else mybir.ActivationFunctionType.Identity)
                        nc.scalar.activation(out=y_sb[:], in_=z_all[:, i, :],
                                             func=func,
                                             bias=nmr_b[:, i:i + 1],
                                             scale=rstd_b[:, i:i + 1])
                        if not simple_k:
                            if cfg["has_gk"]:
                                nc.vector.tensor_mul(out=y_sb[:], in0=y_sb[:],
                                                     in1=gk_b[:])
                            if cfg["has_betak"]:
                                nc.vector.tensor_add(out=y_sb[:], in0=y_sb[:],
                                                     in1=betak_b[:])
                            nc.vector.tensor_scalar_max(out=y_sb[:],
                                                        in0=y_sb[:],
                                                        scalar1=0.0)

                        oh = esb.tile([P, P], bf16, tag="oh")
                        nc.vector.tensor_scalar(out=oh[:], in0=iota_t[:],
                                                scalar1=dloc_w[:, s:s + 1],
                                                scalar2=None,
                                                op0=mybir.AluOpType.is_equal)
                        nc.tensor.matmul(out=acc_ps[:], lhsT=oh[:],
                                         rhs=y_sb[:], start=(s == 0),
                                         stop=(s == K - 1),
                                         skip_group_check=True)

                out_sb = esb.tile([P, OUT], f32, tag="out_sb")
                nc.vector.tensor_copy(out=out_sb[:], in_=acc_ps[:])
                nc.sync.dma_start(out=out_part[w * P:(w + 1) * P, :],
                                  in_=out_sb[:])

    nc.compile()
    return nc


# --------------------------------------------------------------------------
# entry point
# --------------------------------------------------------------------------
def _install_trace_hook():
    import sys, types, ctypes, contextlib
    if "antenv.axon_hooks" in sys.modules:
        return
    lib = ctypes.CDLL("/opt/axon/libaxon_pjrt.so")
    lib.axon_start_nrt_profile.argtypes = [ctypes.POINTER(ctypes.c_int64),
                                           ctypes.c_size_t]
    lib.axon_start_nrt_profile.restype = ctypes.c_int64
    lib.axon_stop_nrt_profile.argtypes = [ctypes.c_char_p]
    lib.axon_stop_nrt_profile.restype = ctypes.c_int64

    @contextlib.contextmanager
    def _hook(output_dir, device_ids):
        import jax
        jax.devices()
        if device_ids:
            ids = (ctypes.c_int64 * len(device_ids))(*device_ids)
            rc = lib.axon_start_nrt_profile(ids, len(device_ids))
        else:
            rc = lib.axon_start_nrt_profile(None, 0)
        if rc != 0:
            raise RuntimeError(f"axon_start_nrt_profile rc={rc}")
        try:
            yield
        finally:
            n = lib.axon_stop_nrt_profile(str(output_dir).encode())
            print(f"profile: {n} file(s) -> {output_dir}")

    mod = types.ModuleType("antenv.axon_hooks")
    mod.get_axon_ntff_profile_hook = lambda: _hook
    sys.modules["antenv.axon_hooks"] = mod
    from concourse import bass_utils
    bass_utils.upload_artifacts = lambda tmpdir: "local://skipped"


def kernel(**inputs):
    cfg, in_maps = _prep(**inputs)
    key = (cfg["N"], cfg["GF"], cfg["OUT"], cfg["K"], cfg["e_pc"],
           cfg["has_bn"], cfg["has_gn"], cfg["has_betan"], cfg["has_bk"],
           cfg["has_gk"], cfg["has_betak"])
    if key not in _BUILD_CACHE:
        _BUILD_CACHE[key] = _build(cfg)
    nc = _BUILD_CACHE[key]

    if USE_SIM:
        from concourse import bass_interp
        sim = bass_interp.MultiCoreSim(nc, N_CORES)
        for c in range(N_CORES):
            for name, arr in in_maps[c].items():
                sim.cores[c].tensor(name)[:] = arr
        sim.simulate()
        parts = [np.array(sim.cores[c].tensor("out_part"))
                 for c in range(N_CORES)]
        exec_ns = None
    else:
        kw = {}
        if TRACE:
            _install_trace_hook()
            kw = dict(trace=True, tmpdir=TRACE_DIR)
        res = run_bass_kernel_spmd(nc, in_maps, list(range(N_CORES)), **kw)
        parts = [res.results[c]["out_part"] for c in range(N_CORES)]
        exec_ns = res.exec_time_ns
        kernel.last_exec_ns = exec_ns

    out = np.concatenate(parts, axis=0)[:cfg["N"]]
    return out.astype(np.float32)


kernel.last_exec_ns = None


# revision 10
# speedup vs baseline: 1.0380x; 1.0380x over previous
"""KroneckerMessage GNN message passing on 8 TRN2 NeuronCores.

v2 design (see kernel_v1_baseline.py for the previous version):
- Node phase: shard nodes 8-way; each core computes its slice of
  h = relu(LN(node_feat @ W_node + b_node)) and writes it bf16 into a
  pair-packed table (2 nodes per 256B row, each node padded to 64 bf16);
  AllGather so every core has the full table h_pad [NPAD/2, 128] bf16.
- Edge phase: shard edges by dst range. Host buckets each core's edges
  into 128-node dst windows, padding every window to K subtiles of 128
  edge slots. Per window: ONE gpsimd.dma_gather fetches h rows for all
  src and dst slots (2K*128 indices, idx = node//2 fits int16 because of
  pair packing); a DVE copy + copy_predicated picks the node's half by
  parity. Per 128-edge subtile: DVE builds kron [128,400] bf16 via
  broadcast APs; PE transposes to [100,4x128]; ACT evacuates PSUM; 4
  accumulating bf16 matmuls vs W_kron chunks -> z [128e,128o] f32 in a
  batched PSUM tile; DVE bn_stats/bn_aggr per subtile + per-batch
  sqrt/recip/negmul (batched over B subtiles to amortize small-op cost);
  final ACT relu((z-mu)*rstd) -> y bf16; scatter-add into the window via
  one-hot matmul accumulated in PSUM; flush each window once to DRAM.
"""
import math
import os

import numpy as np
import ml_dtypes

import concourse.bacc as bacc
import concourse.bass as bass
import concourse.tile as tile
from concourse import mybir
from concourse.bass_utils import run_bass_kernel_spmd
from concourse.masks import make_identity

N_CORES = 8
P = 128
D_NODE = 20  # node projection dim
KRON = 400  # 20*20
KCH = 100  # kron rows per matmul chunk
NCH = 4  # kron chunks
LN_EPS = 1e-5
ROW = 128  # bf16 elems per packed pair-row (256B)
HALF = 64  # bf16 elems per node slot inside a pair-row
EXT = 32  # extracted elems per edge slot (>= D_NODE)

# module-level knobs (test.py pokes these)
TRACE = False
TRACE_DIR = None
USE_SIM = False

_BUILD_CACHE = {}


# --------------------------------------------------------------------------
# host-side prep
# --------------------------------------------------------------------------
def _prep(node_feat, W_node, b_node, g_node, beta_node, W_kron, b_kron,
          g_kron, beta_kron, src, dst):
    N, GF = node_feat.shape
    OUT = W_kron.shape[1]
    E = src.shape[0]
    assert GF % P == 0 and OUT == P and W_kron.shape[0] == KRON

    nodes_pc = int(math.ceil(N / (N_CORES * P))) * P
    npad = nodes_pc * N_CORES
    wpc = nodes_pc // P
    assert npad // 2 <= 32767, "pair-packed table must fit int16 indices"

    src = np.asarray(src, np.int64)
    dst = np.asarray(dst, np.int64)
    win = (dst // P).astype(np.int64)  # global window id
    counts = np.bincount(win, minlength=wpc * N_CORES)
    K = max(1, int(math.ceil(counts.max() / P)))
    slots_pw = K * P
    e_pc = wpc * slots_pw

    order = np.argsort(win, kind="stable")
    swin = win[order]
    starts = np.concatenate([[0], np.cumsum(counts)])
    rank = np.arange(E, dtype=np.int64) - starts[swin]
    slot = (swin // wpc) * e_pc + (swin % wpc) * slots_pw + rank

    gsrc = np.zeros(N_CORES * e_pc, np.int16)
    gdst = np.zeros(N_CORES * e_pc, np.int16)
    par_src = np.zeros(N_CORES * e_pc, np.float32)
    par_dst = np.zeros(N_CORES * e_pc, np.float32)
    dloc = np.full(N_CORES * e_pc, -1.0, np.float32)
    gsrc[slot] = (src[order] // 2).astype(np.int16)
    gdst[slot] = (dst[order] // 2).astype(np.int16)
    par_src[slot] = (src[order] % 2).astype(np.float32)
    par_dst[slot] = (dst[order] % 2).astype(np.float32)
    dloc[slot] = (dst[order] % P).astype(np.float32)

    # gather index tensor per (core, window): [128, 2*slots_pw/16] int16,
    # position j (= s'*128 + p) wrapped to (partition j%16, col j//16),
    # replicated 8x across partition groups.
    idxc = 2 * slots_pw // 16
    gidx = np.zeros((N_CORES, wpc, 128, idxc), np.int16)
    par = np.zeros((N_CORES, wpc, 2 * slots_pw), np.float32)
    for c in range(N_CORES):
        for w in range(wpc):
            b0 = c * e_pc + w * slots_pw
            lst = np.concatenate([gsrc[b0:b0 + slots_pw], gdst[b0:b0 + slots_pw]])
            wrapped = lst.reshape(-1, 16).T  # [16, idxc]
            gidx[c, w] = np.tile(wrapped, (8, 1))
            par[c, w] = np.concatenate(
                [par_src[b0:b0 + slots_pw], par_dst[b0:b0 + slots_pw]])

    nf_pad = np.zeros((npad, GF), np.float32)
    nf_pad[:N] = np.asarray(node_feat, np.float32)

    # W_node -> [P, GF//P, D_NODE]
    wn = np.asarray(W_node, np.float32).reshape(GF // P, P, D_NODE)
    wn = np.ascontiguousarray(wn.transpose(1, 0, 2))

    # W_kron chunks: [KCH, NCH, OUT] bf16
    wk = np.asarray(W_kron, np.float32).reshape(NCH, KCH, OUT)
    wk = np.ascontiguousarray(wk.transpose(1, 0, 2))

    flags = dict(
        has_bn=bool(np.any(np.asarray(b_node) != 0)),
        has_gn=bool(np.any(np.asarray(g_node) != 1)),
        has_betan=bool(np.any(np.asarray(beta_node) != 0)),
        has_bk=bool(np.any(np.asarray(b_kron) != 0)),
        has_gk=bool(np.any(np.asarray(g_kron) != 1)),
        has_betak=bool(np.any(np.asarray(beta_kron) != 0)),
    )

    cfg = dict(N=N, GF=GF, OUT=OUT, E=E, nodes_pc=nodes_pc, npad=npad,
               wpc=wpc, K=K, e_pc=e_pc, **flags)

    bf = ml_dtypes.bfloat16
    in_maps = []
    for c in range(N_CORES):
        m = dict(
            nf=nf_pad[c * nodes_pc:(c + 1) * nodes_pc],
            wn=wn,
            wk=wk.astype(bf),
            gidx=gidx[c],
            par=par[c].reshape(-1).astype(np.uint8),
            dloc=dloc[c * e_pc:(c + 1) * e_pc],
            b_node=np.asarray(b_node, np.float32),
            g_node=np.asarray(g_node, np.float32),
            beta_node=np.asarray(beta_node, np.float32),
            bk=np.asarray(b_kron, np.float32).astype(bf),
            g_kron=np.asarray(g_kron, np.float32).astype(bf),
            beta_kron=np.asarray(beta_kron, np.float32).astype(bf),
            iota_bf=np.arange(P, dtype=np.float32).astype(bf),
        )
        in_maps.append(m)
    return cfg, in_maps


# --------------------------------------------------------------------------
# device program
# --------------------------------------------------------------------------
def _build(cfg):
    GF, OUT = cfg["GF"], cfg["OUT"]
    nodes_pc, npad, wpc, K, e_pc = (cfg["nodes_pc"], cfg["npad"], cfg["wpc"],
                                    cfg["K"], cfg["e_pc"])
    FCH = GF // P
    f32, bf16, i16 = mybir.dt.float32, mybir.dt.bfloat16, mybir.dt.int16
    slots_pw = K * P
    idxc = 2 * slots_pw // 16
    # LN batch size (PSUM budget: z tile is B*512B, <= 1.5 banks at B=6)
    B = min(6, K)
    NB = int(math.ceil(K / B))

    nc = bacc.Bacc(num_devices=N_CORES, num_swdge_queues=4)
    nf = nc.dram_tensor("nf", [nodes_pc, GF], f32, kind="ExternalInput")
    wn = nc.dram_tensor("wn", [P, FCH, D_NODE], f32, kind="ExternalInput")
    wk = nc.dram_tensor("wk", [KCH, NCH, OUT], bf16, kind="ExternalInput")
    gidx = nc.dram_tensor("gidx", [wpc, 128, idxc], i16, kind="ExternalInput")
    par = nc.dram_tensor("par", [wpc * 2 * slots_pw], mybir.dt.uint8,
                         kind="ExternalInput")
    dloc = nc.dram_tensor("dloc", [e_pc], f32, kind="ExternalInput")
    b_node = nc.dram_tensor("b_node", [D_NODE], f32, kind="ExternalInput")
    g_node = nc.dram_tensor("g_node", [D_NODE], f32, kind="ExternalInput")
    beta_node = nc.dram_tensor("beta_node", [D_NODE], f32, kind="ExternalInput")
    bk = nc.dram_tensor("bk", [OUT], bf16, kind="ExternalInput")
    g_kron = nc.dram_tensor("g_kron", [OUT], bf16, kind="ExternalInput")
    beta_kron = nc.dram_tensor("beta_kron", [OUT], bf16, kind="ExternalInput")
    iota_bf = nc.dram_tensor("iota_bf", [P], bf16, kind="ExternalInput")

    out_part = nc.dram_tensor("out_part", [nodes_pc, OUT], f32,
                              kind="ExternalOutput")
    h_pad_part = nc.dram_tensor("h_pad_part", [nodes_pc // 2, ROW], bf16)
    h_pad_full = nc.dram_tensor("h_pad_full", [npad // 2, ROW], bf16,
                                addr_space="Shared")

    ntiles = nodes_pc // P

    # ---------------- phase 1: h = relu(LN(nf @ W_node + b)) --------------
    with tile.TileContext(nc) as tc:
        with (
            tc.tile_pool(name="hconst", bufs=1) as hconst,
            tc.tile_pool(name="hsb", bufs=3) as hsb,
            tc.tile_pool(name="hps", bufs=2, space="PSUM") as hps,
            tc.tile_pool(name="hsmall", bufs=4) as hsmall,
        ):
            ident_f32 = hconst.tile([P, P], f32)
            make_identity(nc, ident_f32[:])
            wn_sb = hconst.tile([P, FCH, D_NODE], f32)
            nc.gpsimd.dma_start(out=wn_sb[:], in_=wn[:])
            eps_t = hconst.tile([P, 1], f32)
            nc.vector.memset(eps_t[:], LN_EPS)
            if cfg["has_bn"]:
                bn_b = hconst.tile([P, D_NODE], f32)
                nc.gpsimd.dma_start(
                    out=bn_b[:],
                    in_=bass.AP(tensor=b_node, offset=0,
                                ap=[[0, P], [1, D_NODE]]))
            if cfg["has_gn"]:
                gn_b = hconst.tile([P, D_NODE], f32)
                nc.gpsimd.dma_start(
                    out=gn_b[:],
                    in_=bass.AP(tensor=g_node, offset=0,
                                ap=[[0, P], [1, D_NODE]]))
            if cfg["has_betan"]:
                betan_b = hconst.tile([P, D_NODE], f32)
                nc.gpsimd.dma_start(
                    out=betan_b[:],
                    in_=bass.AP(tensor=beta_node, offset=0,
                                ap=[[0, P], [1, D_NODE]]))

            h_stage = hconst.tile([P, ntiles, HALF], bf16)
            nc.vector.memset(h_stage[:], 0.0)

            for t in range(ntiles):
                nf_t = hsb.tile([P, GF], f32, tag="nf_t")
                nc.sync.dma_start(out=nf_t[:], in_=nf[t * P:(t + 1) * P, :])
                nfT_ps = hps.tile([P, FCH, P], f32, tag="nfT_ps")
                for c in range(FCH):
                    nc.tensor.transpose(out=nfT_ps[:, c, :],
                                        in_=nf_t[:, c * P:(c + 1) * P],
                                        identity=ident_f32[:])
                nfT = hsb.tile([P, FCH, P], f32, tag="nfT")
                nc.vector.tensor_copy(out=nfT[:], in_=nfT_ps[:])
                z_ps = hps.tile([P, D_NODE], f32, tag="z_ps")
                for c in range(FCH):
                    nc.tensor.matmul(out=z_ps[:], lhsT=nfT[:, c, :],
                                     rhs=wn_sb[:, c, :], start=(c == 0),
                                     stop=(c == FCH - 1))
                if cfg["has_bn"]:
                    z_sb = hsb.tile([P, D_NODE], f32, tag="z_sb")
                    nc.vector.tensor_add(out=z_sb[:], in0=z_ps[:], in1=bn_b[:])
                    z_in = z_sb
                else:
                    z_in = z_ps
                stats = hsmall.tile([P, 6], f32, tag="stats")
                nc.vector.bn_stats(out=stats[:], in_=z_in[:])
                mv = hsmall.tile([P, 2], f32, tag="mv")
                nc.vector.bn_aggr(out=mv[:], in_=stats[:])
                sd = hsmall.tile([P, 1], f32, tag="sd")
                nc.scalar.activation(out=sd[:], in_=mv[:, 1:2],
                                     func=mybir.ActivationFunctionType.Sqrt,
                                     bias=eps_t[:], scale=1.0)
                rstd = hsmall.tile([P, 1], f32, tag="rstd")
                nc.vector.reciprocal(out=rstd[:], in_=sd[:])
                nmr = hsmall.tile([P, 1], f32, tag="nmr")
                nc.vector.tensor_scalar(out=nmr[:], in0=mv[:, 0:1],
                                        scalar1=rstd[:, 0:1], scalar2=-1.0,
                                        op0=mybir.AluOpType.mult,
                                        op1=mybir.AluOpType.mult)
                simple = not (cfg["has_gn"] or cfg["has_betan"])
                func = (mybir.ActivationFunctionType.Relu if simple
                        else mybir.ActivationFunctionType.Identity)
                nc.scalar.activation(out=h_stage[:, t, 0:D_NODE], in_=z_in[:],
                                     func=func, bias=nmr[:],
                                     scale=rstd[:, 0:1])
                if not simple:
                    if cfg["has_gn"]:
                        nc.vector.tensor_mul(out=h_stage[:, t, 0:D_NODE],
                                             in0=h_stage[:, t, 0:D_NODE],
                                             in1=gn_b[:])
                    if cfg["has_betan"]:
                        nc.vector.tensor_add(out=h_stage[:, t, 0:D_NODE],
                                             in0=h_stage[:, t, 0:D_NODE],
                                             in1=betan_b[:])
                    nc.vector.tensor_scalar_max(out=h_stage[:, t, 0:D_NODE],
                                                in0=h_stage[:, t, 0:D_NODE],
                                                scalar1=0.0)
            # node n = t*128 + p lands in pair-row t*64 + p//2, half p%2
            nc.sync.dma_start(
                out=h_pad_part.rearrange("(t p2) (pm j) -> (p2 pm) t j",
                                         p2=P // 2, pm=2, j=HALF),
                in_=h_stage[:])

    # ---------------- collective: AllGather h ----------------------------
    with (
        nc.Block() as block,
        nc.semaphore("cc_sem") as cc_sem,
    ):
        @block.gpsimd
        def _(gpsimd):
            gpsimd.collective_compute(
                "AllGather",
                mybir.AluOpType.bypass,
                replica_groups=[list(range(N_CORES))],
                ins=[h_pad_part[:]],
                outs=[h_pad_full[:]],
            ).then_inc(cc_sem)
            gpsimd.wait_ge(cc_sem, 1)

    # ---------------- phase 2: edges --------------------------------------
    simple_k = not (cfg["has_gk"] or cfg["has_betak"])
    h_rows = bass.AP(tensor=h_pad_full, offset=0,
                     ap=[[ROW, npad // 2], [1, ROW]])
    with tile.TileContext(nc) as tc:
        with (
            tc.tile_pool(name="econst", bufs=1) as econst,
            tc.tile_pool(name="eg", bufs=2) as eg,
            tc.tile_pool(name="esb", bufs=3) as esb,
            tc.tile_pool(name="eps_t", bufs=2, space="PSUM") as epsT,
            tc.tile_pool(name="eps_z", bufs=2, space="PSUM") as epsZ,
            tc.tile_pool(name="eps_a", bufs=2, space="PSUM") as epsA,
            tc.tile_pool(name="esmall", bufs=4) as esmall,
        ):
            ident_bf = econst.tile([P, P], bf16)
            make_identity(nc, ident_bf[:])
            iota_t = econst.tile([P, P], bf16)
            nc.gpsimd.dma_start(
                out=iota_t[:], in_=bass.AP(tensor=iota_bf, offset=0,
                                           ap=[[0, P], [1, P]]))
            eps_t2 = econst.tile([P, 1], f32)
            nc.vector.memset(eps_t2[:], LN_EPS)
            wk_sb = econst.tile([KCH, NCH, OUT], bf16)
            nc.gpsimd.dma_start(out=wk_sb[:], in_=wk[:])
            if cfg["has_bk"]:
                ones_row = econst.tile([1, P], bf16)
                nc.vector.memset(ones_row[:], 1.0)
                bk_sb = econst.tile([1, OUT], bf16)
                nc.gpsimd.dma_start(out=bk_sb[:], in_=bk[None, :])
            if cfg["has_gk"]:
                gk_b = econst.tile([P, OUT], bf16)
                nc.gpsimd.dma_start(
                    out=gk_b[:], in_=bass.AP(tensor=g_kron, offset=0,
                                             ap=[[0, P], [1, OUT]]))
            if cfg["has_betak"]:
                betak_b = econst.tile([P, OUT], bf16)
                nc.gpsimd.dma_start(
                    out=betak_b[:],
                    in_=bass.AP(tensor=beta_kron, offset=0,
                                ap=[[0, P], [1, OUT]]))

            for w in range(wpc):
                base = w * slots_pw
                dloc_w = eg.tile([P, K], f32, tag="dloc_w")
                nc.sync.dma_start(
                    out=dloc_w[:],
                    in_=bass.AP(tensor=dloc, offset=base,
                                ap=[[1, P], [P, K]]))
                par_w = eg.tile([P, 2 * K], mybir.dt.uint8, tag="par_w")
                nc.sync.dma_start(
                    out=par_w[:],
                    in_=bass.AP(tensor=par, offset=2 * base,
                                ap=[[1, P], [P, 2 * K]]))
                gidx_w = eg.tile([128, idxc], i16, tag="gidx_w")
                nc.sync.dma_start(out=gidx_w[:], in_=gidx[w])

                graw = eg.tile([P, 2 * K, ROW], bf16, tag="graw")
                nc.gpsimd.dma_gather(
                    graw[:], h_rows, gidx_w[:], 2 * slots_pw, 2 * slots_pw,
                    ROW, elem_step=ROW, single_packet=False,
                    queue_num=w % 4)

                # pick node half by parity: hsel[:, s', :] =
                #   graw[:, s', par?64+ : 0+][0:EXT]
                # inner dim padded to 34 so the out AP stays 3-D (the sim's
                # np.where needs out/mask/data ndim to line up)
                hsel = eg.tile([P, 2 * K, EXT + 2], bf16, tag="hsel")
                nc.vector.tensor_copy(out=hsel[:, :, 0:EXT],
                                      in_=graw[:, :, 0:EXT])
                nc.vector.copy_predicated(
                    out=hsel[:, :, 0:EXT],
                    mask=par_w[:, :, None].to_broadcast([P, 2 * K, EXT]),
                    data=graw[:, :, HALF:HALF + EXT])

                acc_ps = epsA.tile([P, OUT], f32, tag="acc")

                for b in range(NB):
                    nsub = min(B, K - b * B)
                    z_all = epsZ.tile([P, B, OUT], f32, tag="z_all")
                    mv_all = esmall.tile([P, B, 2], f32, tag="mv_all")
                    for i in range(nsub):
                        s = b * B + i
                        kron = esb.tile([P, D_NODE, D_NODE], bf16, tag="kron")
                        nc.vector.tensor_tensor(
                            out=kron[:],
                            in0=hsel[:, s, 0:D_NODE, None].to_broadcast(
                                [P, D_NODE, D_NODE]),
                            in1=hsel[:, K + s, None, 0:D_NODE].to_broadcast(
                                [P, D_NODE, D_NODE]),
                            op=mybir.AluOpType.mult)
                        kv = kron[:].rearrange("p a b -> p (a b)")
                        psT = epsT.tile([KCH, NCH, P], bf16, tag="psT")
                        for c in range(NCH):
                            nc.tensor.transpose(
                                out=psT[:, c, :],
                                in_=kv[:, c * KCH:(c + 1) * KCH],
                                identity=ident_bf[:])
                        kron_sb = esb.tile([KCH, NCH, P], bf16, tag="kron_sb")
                        nc.scalar.activation(
                            out=kron_sb[:], in_=psT[:],
                            func=mybir.ActivationFunctionType.Copy)
                        zi = z_all[:, i, :]
                        nmm = NCH + (1 if cfg["has_bk"] else 0)
                        for c in range(NCH):
                            nc.tensor.matmul(out=zi, lhsT=kron_sb[:, c, :],
                                             rhs=wk_sb[:, c, :],
                                             start=(c == 0),
                                             stop=(c == nmm - 1))
                        if cfg["has_bk"]:
                            nc.tensor.matmul(out=zi, lhsT=ones_row[:],
                                             rhs=bk_sb[:], start=False,
                                             stop=True, skip_group_check=True)
                        stats = esmall.tile([P, 6], f32, tag="stats")
                        nc.vector.bn_stats(out=stats[:], in_=zi)
                        nc.vector.bn_aggr(out=mv_all[:, i, :], in_=stats[:])

                    # batched LN scalars over the B subtiles
                    sd_b = esmall.tile([P, B], f32, tag="sd_b")
                    nc.scalar.activation(
                        out=sd_b[:, 0:nsub], in_=mv_all[:, 0:nsub, 1],
                        func=mybir.ActivationFunctionType.Sqrt,
                        bias=eps_t2[:], scale=1.0)
                    rstd_b = esmall.tile([P, B], f32, tag="rstd_b")
                    nc.vector.reciprocal(out=rstd_b[:, 0:nsub],
                                         in_=sd_b[:, 0:nsub])
                    nmr_b = esmall.tile([P, B], f32, tag="nmr_b")
                    nc.vector.scalar_tensor_tensor(
                        out=nmr_b[:, 0:nsub], in0=mv_all[:, 0:nsub, 0],
                        scalar=-1.0, in1=rstd_b[:, 0:nsub],
                        op0=mybir.AluOpType.mult, op1=mybir.AluOpType.mult)

                    for i in range(nsub):
                        s = b * B + i
                        y_sb = esb.tile([P, OUT], bf16, tag="y")
                        func = (mybir.ActivationFunctionType.Relu if simple_k
                                else mybir.ActivationFunctionType.Identity)
                        nc.scalar.activation(out=y_sb[:], in_=z_all[:, i, :],
                                             func=func,
                                             bias=nmr_b[:, i:i + 1],
                                             scale=rstd_b[:, i:i + 1])
                        if not simple_k:
                            if cfg["has_gk"]:
                                nc.vector.tensor_mul(out=y_sb[:], in0=y_sb[:],
                                                     in1=gk_b[:])
                            if cfg["has_betak"]:
                                nc.vector.tensor_add(out=y_sb[:], in0=y_sb[:],
                                                     in1=betak_b[:])
                            nc.vector.tensor_scalar_max(out=y_sb[:],
                                                        in0=y_sb[:],
                                                        scalar1=0.0)

                        oh = esb.tile([P, P], bf16, tag="oh")
                        nc.vector.tensor_scalar(out=oh[:], in0=iota_t[:],
                                                scalar1=dloc_w[:, s:s + 1],
                                                scalar2=None,
                                                op0=mybir.AluOpType.is_equal)
                        nc.tensor.matmul(out=acc_ps[:], lhsT=oh[:],
                                         rhs=y_sb[:], start=(s == 0),
                                         stop=(s == K - 1),
                                         skip_group_check=True)

                out_sb = esb.tile([P, OUT], f32, tag="out_sb")
                nc.vector.tensor_copy(out=out_sb[:], in_=acc_ps[:])
                nc.sync.dma_start(out=out_part[w * P:(w + 1) * P, :],
                                  in_=out_sb[:])

    nc.compile()
    return nc


# --------------------------------------------------------------------------
# entry point
# --------------------------------------------------------------------------
def _install_trace_hook():
    import sys, types, ctypes, contextlib
    if "antenv.axon_hooks" in sys.modules:
        return
    lib = ctypes.CDLL("/opt/axon/libaxon_pjrt.so")
    lib.axon_start_nrt_profile.argtypes = [ctypes.POINTER(ctypes.c_int64),
                                           ctypes.c_size_t]
    lib.axon_start_nrt_profile.restype = ctypes.c_int64
    lib.axon_stop_nrt_profile.argtypes = [ctypes.c_char_p]
    lib.axon_stop_nrt_profile.restype = ctypes.c_int64

    @contextlib.contextmanager
    def _hook(output_dir, device_ids):
        import jax
        jax.devices()
        if device_ids:
            ids = (ctypes.c_int64 * len(device_ids))(*device_ids)
            rc = lib.axon_start_nrt_profile(ids, len(device_ids))
        else:
            rc = lib.axon_start_nrt_profile(None, 0)
        if rc != 0:
            raise RuntimeError(f"axon_start_nrt_profile rc={rc}")
        try:
            yield
        finally:
            n = lib.axon_stop_nrt_profile(str(output_dir).encode())
            print(f"profile: {n} file(s) -> {output_dir}")

    mod = types.ModuleType("antenv.axon_hooks")
    mod.get_axon_ntff_profile_hook = lambda: _hook
    sys.modules["antenv.axon_hooks"] = mod
    from concourse import bass_utils
    bass_utils.upload_artifacts = lambda tmpdir: "local://skipped"


def kernel(**inputs):
    cfg, in_maps = _prep(**inputs)
    key = (cfg["N"], cfg["GF"], cfg["OUT"], cfg["K"], cfg["e_pc"],
           cfg["has_bn"], cfg["has_gn"], cfg["has_betan"], cfg["has_bk"],
           cfg["has_gk"], cfg["has_betak"])
    if key not in _BUILD_CACHE:
        _BUILD_CACHE[key] = _build(cfg)
    nc = _BUILD_CACHE[key]

    if USE_SIM:
        from concourse import bass_interp
        sim = bass_interp.MultiCoreSim(nc, N_CORES)
        for c in range(N_CORES):
            for name, arr in in_maps[c].items():
                sim.cores[c].tensor(name)[:] = arr
        sim.simulate()
        parts = [np.array(sim.cores[c].tensor("out_part"))
                 for c in range(N_CORES)]
        exec_ns = None
    else:
        kw = {}
        if TRACE:
            _install_trace_hook()
            kw = dict(trace=True, tmpdir=TRACE_DIR)
        res = run_bass_kernel_spmd(nc, in_maps, list(range(N_CORES)), **kw)
        parts = [res.results[c]["out_part"] for c in range(N_CORES)]
        exec_ns = res.exec_time_ns
        kernel.last_exec_ns = exec_ns

    out = np.concatenate(parts, axis=0)[:cfg["N"]]
    return out.astype(np.float32)


kernel.last_exec_ns = None


# revision 13
# speedup vs baseline: 1.9362x; 1.8653x over previous
"""KroneckerMessage GNN message passing on 8 TRN2 NeuronCores.

v2 design (see kernel_v1_baseline.py for the previous version):
- Node phase: shard nodes 8-way; each core computes its slice of
  h = relu(LN(node_feat @ W_node + b_node)) and writes it bf16 into a
  pair-packed table (2 nodes per 256B row, each node padded to 64 bf16);
  AllGather so every core has the full table h_pad [NPAD/2, 128] bf16.
- Edge phase: shard edges by dst range. Host buckets each core's edges
  into 128-node dst windows, padding every window to K subtiles of 128
  edge slots. Per window: ONE gpsimd.dma_gather fetches h rows for all
  src and dst slots (2K*128 indices, idx = node//2 fits int16 because of
  pair packing); a DVE copy + copy_predicated picks the node's half by
  parity. Per 128-edge subtile: DVE builds kron [128,400] bf16 via
  broadcast APs; PE transposes to [100,4x128]; ACT evacuates PSUM; 4
  accumulating bf16 matmuls vs W_kron chunks -> z [128e,128o] f32 in a
  batched PSUM tile; DVE bn_stats/bn_aggr per subtile + per-batch
  sqrt/recip/negmul (batched over B subtiles to amortize small-op cost);
  final ACT relu((z-mu)*rstd) -> y bf16; scatter-add into the window via
  one-hot matmul accumulated in PSUM; flush each window once to DRAM.
"""
import math
import os

import numpy as np
import ml_dtypes

import concourse.bacc as bacc
import concourse.bass as bass
import concourse.tile as tile
from concourse import mybir
from concourse.bass_utils import run_bass_kernel_spmd
from concourse.masks import make_identity

N_CORES = 8
P = 128
D_NODE = 20  # node projection dim
KRON = 400  # 20*20
KCH = 100  # kron rows per matmul chunk
NCH = 4  # kron chunks
LN_EPS = 1e-5
ROW = 128  # bf16 elems per packed pair-row (256B)
HALF = 64  # bf16 elems per node slot inside a pair-row
EXT = 32  # extracted elems per edge slot (>= D_NODE)

# module-level knobs (test.py pokes these)
TRACE = False
TRACE_DIR = None
USE_SIM = False

_BUILD_CACHE = {}


# --------------------------------------------------------------------------
# host-side prep
# --------------------------------------------------------------------------
def _prep(node_feat, W_node, b_node, g_node, beta_node, W_kron, b_kron,
          g_kron, beta_kron, src, dst):
    N, GF = node_feat.shape
    OUT = W_kron.shape[1]
    E = src.shape[0]
    assert GF % P == 0 and OUT == P and W_kron.shape[0] == KRON

    nodes_pc = int(math.ceil(N / (N_CORES * P))) * P
    npad = nodes_pc * N_CORES
    wpc = nodes_pc // P
    assert npad // 2 <= 32767, "pair-packed table must fit int16 indices"

    src = np.asarray(src, np.int64)
    dst = np.asarray(dst, np.int64)
    win = (dst // P).astype(np.int64)  # global window id
    counts = np.bincount(win, minlength=wpc * N_CORES)
    K = max(1, int(math.ceil(counts.max() / P)))
    slots_pw = K * P
    e_pc = wpc * slots_pw

    order = np.argsort(win, kind="stable")
    swin = win[order]
    starts = np.concatenate([[0], np.cumsum(counts)])
    rank = np.arange(E, dtype=np.int64) - starts[swin]
    slot = (swin // wpc) * e_pc + (swin % wpc) * slots_pw + rank

    gsrc = np.zeros(N_CORES * e_pc, np.int16)
    gdst = np.zeros(N_CORES * e_pc, np.int16)
    par_src = np.zeros(N_CORES * e_pc, np.float32)
    par_dst = np.zeros(N_CORES * e_pc, np.float32)
    dloc = np.full(N_CORES * e_pc, -1.0, np.float32)
    gsrc[slot] = (src[order] // 2).astype(np.int16)
    gdst[slot] = (dst[order] // 2).astype(np.int16)
    par_src[slot] = (src[order] % 2).astype(np.float32)
    par_dst[slot] = (dst[order] % 2).astype(np.float32)
    dloc[slot] = (dst[order] % P).astype(np.float32)

    # gather index tensor per (core, window): [128, 2*slots_pw/16] int16,
    # position j (= s'*128 + p) wrapped to (partition j%16, col j//16),
    # replicated 8x across partition groups.
    idxc = slots_pw // 16
    gidx = np.zeros((N_CORES, wpc, 128, idxc), np.int16)
    for c in range(N_CORES):
        for w in range(wpc):
            b0 = c * e_pc + w * slots_pw
            wrapped = gsrc[b0:b0 + slots_pw].reshape(-1, 16).T  # [16, idxc]
            gidx[c, w] = np.tile(wrapped, (8, 1))
    par = par_src

    nf_pad = np.zeros((npad, GF), np.float32)
    nf_pad[:N] = np.asarray(node_feat, np.float32)

    # W_node -> [P, GF//P, D_NODE]
    wn = np.asarray(W_node, np.float32).reshape(GF // P, P, D_NODE)
    wn = np.ascontiguousarray(wn.transpose(1, 0, 2))

    # W_kron chunks: [KCH, NCH, OUT] bf16
    wk = np.asarray(W_kron, np.float32).reshape(NCH, KCH, OUT)
    wk = np.ascontiguousarray(wk.transpose(1, 0, 2))

    flags = dict(
        has_bn=bool(np.any(np.asarray(b_node) != 0)),
        has_gn=bool(np.any(np.asarray(g_node) != 1)),
        has_betan=bool(np.any(np.asarray(beta_node) != 0)),
        has_bk=bool(np.any(np.asarray(b_kron) != 0)),
        has_gk=bool(np.any(np.asarray(g_kron) != 1)),
        has_betak=bool(np.any(np.asarray(beta_kron) != 0)),
    )

    cfg = dict(N=N, GF=GF, OUT=OUT, E=E, nodes_pc=nodes_pc, npad=npad,
               wpc=wpc, K=K, e_pc=e_pc, **flags)

    bf = ml_dtypes.bfloat16
    in_maps = []
    for c in range(N_CORES):
        m = dict(
            nf=nf_pad[c * nodes_pc:(c + 1) * nodes_pc],
            wn=wn,
            wk=wk.astype(bf),
            gidx=gidx[c],
            par=par[c * e_pc:(c + 1) * e_pc].astype(np.uint8),
            dloc=dloc[c * e_pc:(c + 1) * e_pc].astype(bf),
            iota_f32=np.arange(P, dtype=np.float32),
            b_node=np.asarray(b_node, np.float32),
            g_node=np.asarray(g_node, np.float32),
            beta_node=np.asarray(beta_node, np.float32),
            bk=np.asarray(b_kron, np.float32).astype(bf),
            g_kron=np.asarray(g_kron, np.float32).astype(bf),
            beta_kron=np.asarray(beta_kron, np.float32).astype(bf),
            iota_bf=np.arange(P, dtype=np.float32).astype(bf),
        )
        in_maps.append(m)
    return cfg, in_maps


# --------------------------------------------------------------------------
# device program
# --------------------------------------------------------------------------
def _build(cfg):
    GF, OUT = cfg["GF"], cfg["OUT"]
    nodes_pc, npad, wpc, K, e_pc = (cfg["nodes_pc"], cfg["npad"], cfg["wpc"],
                                    cfg["K"], cfg["e_pc"])
    FCH = GF // P
    f32, bf16, i16 = mybir.dt.float32, mybir.dt.bfloat16, mybir.dt.int16
    slots_pw = K * P
    idxc = slots_pw // 16
    # LN batch size (PSUM budget: z tile is B*512B = 1 bank at B=4)
    B = min(4, K)
    NB = int(math.ceil(K / B))

    nc = bacc.Bacc(num_devices=N_CORES, num_swdge_queues=4)
    nf = nc.dram_tensor("nf", [nodes_pc, GF], f32, kind="ExternalInput")
    wn = nc.dram_tensor("wn", [P, FCH, D_NODE], f32, kind="ExternalInput")
    wk = nc.dram_tensor("wk", [KCH, NCH, OUT], bf16, kind="ExternalInput")
    gidx = nc.dram_tensor("gidx", [wpc, 128, idxc], i16, kind="ExternalInput")
    par = nc.dram_tensor("par", [e_pc], mybir.dt.uint8,
                         kind="ExternalInput")
    dloc = nc.dram_tensor("dloc", [e_pc], bf16, kind="ExternalInput")
    iota_f32 = nc.dram_tensor("iota_f32", [P], f32, kind="ExternalInput")
    b_node = nc.dram_tensor("b_node", [D_NODE], f32, kind="ExternalInput")
    g_node = nc.dram_tensor("g_node", [D_NODE], f32, kind="ExternalInput")
    beta_node = nc.dram_tensor("beta_node", [D_NODE], f32, kind="ExternalInput")
    bk = nc.dram_tensor("bk", [OUT], bf16, kind="ExternalInput")
    g_kron = nc.dram_tensor("g_kron", [OUT], bf16, kind="ExternalInput")
    beta_kron = nc.dram_tensor("beta_kron", [OUT], bf16, kind="ExternalInput")
    iota_bf = nc.dram_tensor("iota_bf", [P], bf16, kind="ExternalInput")

    out_part = nc.dram_tensor("out_part", [nodes_pc, OUT], f32,
                              kind="ExternalOutput")
    h_pad_part = nc.dram_tensor("h_pad_part", [nodes_pc // 2, ROW], bf16)
    h_pad_full = nc.dram_tensor("h_pad_full", [npad // 2, ROW], bf16,
                                addr_space="Shared")

    ntiles = nodes_pc // P

    # ---------------- phase 1: h = relu(LN(nf @ W_node + b)) --------------
    with tile.TileContext(nc) as tc:
        with (
            tc.tile_pool(name="hconst", bufs=1) as hconst,
            tc.tile_pool(name="hsb", bufs=3) as hsb,
            tc.tile_pool(name="hps", bufs=2, space="PSUM") as hps,
            tc.tile_pool(name="hsmall", bufs=4) as hsmall,
        ):
            ident_f32 = hconst.tile([P, P], f32)
            make_identity(nc, ident_f32[:])
            wn_sb = hconst.tile([P, FCH, D_NODE], f32)
            nc.gpsimd.dma_start(out=wn_sb[:], in_=wn[:])
            eps_t = hconst.tile([P, 1], f32)
            nc.vector.memset(eps_t[:], LN_EPS)
            if cfg["has_bn"]:
                bn_b = hconst.tile([P, D_NODE], f32)
                nc.gpsimd.dma_start(
                    out=bn_b[:],
                    in_=bass.AP(tensor=b_node, offset=0,
                                ap=[[0, P], [1, D_NODE]]))
            if cfg["has_gn"]:
                gn_b = hconst.tile([P, D_NODE], f32)
                nc.gpsimd.dma_start(
                    out=gn_b[:],
                    in_=bass.AP(tensor=g_node, offset=0,
                                ap=[[0, P], [1, D_NODE]]))
            if cfg["has_betan"]:
                betan_b = hconst.tile([P, D_NODE], f32)
                nc.gpsimd.dma_start(
                    out=betan_b[:],
                    in_=bass.AP(tensor=beta_node, offset=0,
                                ap=[[0, P], [1, D_NODE]]))

            h_stage = hconst.tile([P, ntiles, HALF], bf16)
            nc.vector.memset(h_stage[:], 0.0)

            for t in range(ntiles):
                nf_t = hsb.tile([P, GF], f32, tag="nf_t")
                nc.sync.dma_start(out=nf_t[:], in_=nf[t * P:(t + 1) * P, :])
                nfT_ps = hps.tile([P, FCH, P], f32, tag="nfT_ps")
                for c in range(FCH):
                    nc.tensor.transpose(out=nfT_ps[:, c, :],
                                        in_=nf_t[:, c * P:(c + 1) * P],
                                        identity=ident_f32[:])
                nfT = hsb.tile([P, FCH, P], f32, tag="nfT")
                nc.vector.tensor_copy(out=nfT[:], in_=nfT_ps[:])
                z_ps = hps.tile([P, D_NODE], f32, tag="z_ps")
                for c in range(FCH):
                    nc.tensor.matmul(out=z_ps[:], lhsT=nfT[:, c, :],
                                     rhs=wn_sb[:, c, :], start=(c == 0),
                                     stop=(c == FCH - 1))
                if cfg["has_bn"]:
                    z_sb = hsb.tile([P, D_NODE], f32, tag="z_sb")
                    nc.vector.tensor_add(out=z_sb[:], in0=z_ps[:], in1=bn_b[:])
                    z_in = z_sb
                else:
                    z_in = z_ps
                stats = hsmall.tile([P, 6], f32, tag="stats")
                nc.vector.bn_stats(out=stats[:], in_=z_in[:])
                mv = hsmall.tile([P, 2], f32, tag="mv")
                nc.vector.bn_aggr(out=mv[:], in_=stats[:])
                sd = hsmall.tile([P, 1], f32, tag="sd")
                nc.scalar.activation(out=sd[:], in_=mv[:, 1:2],
                                     func=mybir.ActivationFunctionType.Sqrt,
                                     bias=eps_t[:], scale=1.0)
                rstd = hsmall.tile([P, 1], f32, tag="rstd")
                nc.vector.reciprocal(out=rstd[:], in_=sd[:])
                nmr = hsmall.tile([P, 1], f32, tag="nmr")
                nc.vector.tensor_scalar(out=nmr[:], in0=mv[:, 0:1],
                                        scalar1=rstd[:, 0:1], scalar2=-1.0,
                                        op0=mybir.AluOpType.mult,
                                        op1=mybir.AluOpType.mult)
                simple = not (cfg["has_gn"] or cfg["has_betan"])
                func = (mybir.ActivationFunctionType.Relu if simple
                        else mybir.ActivationFunctionType.Identity)
                nc.scalar.activation(out=h_stage[:, t, 0:D_NODE], in_=z_in[:],
                                     func=func, bias=nmr[:],
                                     scale=rstd[:, 0:1])
                if not simple:
                    if cfg["has_gn"]:
                        nc.vector.tensor_mul(out=h_stage[:, t, 0:D_NODE],
                                             in0=h_stage[:, t, 0:D_NODE],
                                             in1=gn_b[:])
                    if cfg["has_betan"]:
                        nc.vector.tensor_add(out=h_stage[:, t, 0:D_NODE],
                                             in0=h_stage[:, t, 0:D_NODE],
                                             in1=betan_b[:])
                    nc.vector.tensor_scalar_max(out=h_stage[:, t, 0:D_NODE],
                                                in0=h_stage[:, t, 0:D_NODE],
                                                scalar1=0.0)
            # node n = t*128 + p lands in pair-row t*64 + p//2, half p%2
            nc.sync.dma_start(
                out=h_pad_part.rearrange("(t p2) (pm j) -> (p2 pm) t j",
                                         p2=P // 2, pm=2, j=HALF),
                in_=h_stage[:])

    # ---------------- collective: AllGather h ----------------------------
    with (
        nc.Block() as block,
        nc.semaphore("cc_sem") as cc_sem,
    ):
        @block.gpsimd
        def _(gpsimd):
            gpsimd.collective_compute(
                "AllGather",
                mybir.AluOpType.bypass,
                replica_groups=[list(range(N_CORES))],
                ins=[h_pad_part[:]],
                outs=[h_pad_full[:]],
            ).then_inc(cc_sem)
            gpsimd.wait_ge(cc_sem, 1)

    # ---------------- phase 2: edges --------------------------------------
    simple_k = not (cfg["has_gk"] or cfg["has_betak"])
    h_rows = bass.AP(tensor=h_pad_full, offset=0,
                     ap=[[ROW, npad // 2], [1, ROW]])
    with tile.TileContext(nc) as tc:
        with (
            tc.tile_pool(name="econst", bufs=1) as econst,
            tc.tile_pool(name="eg", bufs=2) as eg,
            tc.tile_pool(name="esb", bufs=3) as esb,
            tc.tile_pool(name="eps_t", bufs=2, space="PSUM") as epsT,
            tc.tile_pool(name="eps_z", bufs=2, space="PSUM") as epsZ,
            tc.tile_pool(name="eps_a", bufs=2, space="PSUM") as epsA,
            tc.tile_pool(name="eps_h", bufs=2, space="PSUM") as epsH,
            tc.tile_pool(name="esmall", bufs=4) as esmall,
        ):
            ident_bf = econst.tile([P, P], bf16)
            make_identity(nc, ident_bf[:])
            iota_t = econst.tile([P, P], bf16)
            nc.gpsimd.dma_start(
                out=iota_t[:], in_=bass.AP(tensor=iota_bf, offset=0,
                                           ap=[[0, P], [1, P]]))
            iota_col = econst.tile([P, 1], f32)
            nc.gpsimd.dma_start(
                out=iota_col[:], in_=bass.AP(tensor=iota_f32, offset=0,
                                             ap=[[1, P], [1, 1]]))
            eps_t2 = econst.tile([P, 1], f32)
            nc.vector.memset(eps_t2[:], LN_EPS)
            wk_sb = econst.tile([KCH, NCH, OUT], bf16)
            nc.gpsimd.dma_start(out=wk_sb[:], in_=wk[:])
            if cfg["has_bk"]:
                ones_row = econst.tile([1, P], bf16)
                nc.vector.memset(ones_row[:], 1.0)
                bk_sb = econst.tile([1, OUT], bf16)
                nc.gpsimd.dma_start(out=bk_sb[:], in_=bk[None, :])
            if cfg["has_gk"]:
                gk_b = econst.tile([P, OUT], bf16)
                nc.gpsimd.dma_start(
                    out=gk_b[:], in_=bass.AP(tensor=g_kron, offset=0,
                                             ap=[[0, P], [1, OUT]]))
            if cfg["has_betak"]:
                betak_b = econst.tile([P, OUT], bf16)
                nc.gpsimd.dma_start(
                    out=betak_b[:],
                    in_=bass.AP(tensor=beta_kron, offset=0,
                                ap=[[0, P], [1, OUT]]))

            for w in range(wpc):
                base = w * slots_pw
                # bf16 dst-local ids, [slot-in-subtile, subtile] layout
                dloc_w = eg.tile([P, K], bf16, tag="dloc_w")
                nc.sync.dma_start(
                    out=dloc_w[:],
                    in_=bass.AP(tensor=dloc, offset=base,
                                ap=[[1, P], [P, K]]))
                # same values replicated to all partitions: [n, s, e]
                dloc_b = eg.tile([P, K, P], bf16, tag="dloc_b")
                nc.sync.dma_start(
                    out=dloc_b[:],
                    in_=bass.AP(tensor=dloc, offset=base,
                                ap=[[0, P], [P, K], [1, P]]))
                par_w = eg.tile([P, K], mybir.dt.uint8, tag="par_w")
                nc.sync.dma_start(
                    out=par_w[:],
                    in_=bass.AP(tensor=par, offset=base,
                                ap=[[1, P], [P, K]]))
                gidx_w = eg.tile([128, idxc], i16, tag="gidx_w")
                nc.sync.dma_start(out=gidx_w[:], in_=gidx[w])
                # this core's own h rows for the window (dst side is dense)
                hwin = eg.tile([P, EXT], bf16, tag="hwin")
                nc.sync.dma_start(
                    out=hwin[:],
                    in_=bass.AP(tensor=h_pad_part, offset=w * (P // 2) * ROW,
                                ap=[[ROW, P // 2], [HALF, 2], [1, EXT]]))

                graw = eg.tile([P, K, ROW], bf16, tag="graw")
                nc.gpsimd.dma_gather(
                    graw[:], h_rows, gidx_w[:], slots_pw, slots_pw,
                    ROW, elem_step=ROW, single_packet=False)

                # pick node half by parity: hsel[:, s, :] =
                #   graw[:, s, par?64+ : 0+][0:EXT]
                # inner dim padded to 34 so the out AP stays 3-D (the sim's
                # np.where needs out/mask/data ndim to line up)
                hsel = eg.tile([P, K, EXT + 2], bf16, tag="hsel")
                nc.vector.tensor_copy(out=hsel[:, :, 0:EXT],
                                      in_=graw[:, :, 0:EXT])
                nc.vector.copy_predicated(
                    out=hsel[:, :, 0:EXT],
                    mask=par_w[:, :, None].to_broadcast([P, K, EXT]),
                    data=graw[:, :, HALF:HALF + EXT])

                # batched one-hots for the whole window:
                # oh_all[e, s, n] = (dloc[s,e] == n); ohT_all[n, s, e] = same.T
                oh_all = eg.tile([P, K, P], bf16, tag="oh_all")
                nc.vector.tensor_tensor(
                    out=oh_all[:],
                    in0=iota_t[:, None, :].to_broadcast([P, K, P]),
                    in1=dloc_w[:, :, None].to_broadcast([P, K, P]),
                    op=mybir.AluOpType.is_equal)
                ohT_all = eg.tile([P, K, P], bf16, tag="ohT_all")
                nc.vector.tensor_scalar(
                    out=ohT_all[:], in0=dloc_b[:],
                    scalar1=iota_col[:, 0:1], scalar2=None,
                    op0=mybir.AluOpType.is_equal)

                acc_ps = epsA.tile([P, OUT], f32, tag="acc")

                for b in range(NB):
                    nsub = min(B, K - b * B)
                    z_all = epsZ.tile([P, B, OUT], f32, tag="z_all")
                    mv_all = esmall.tile([P, B, 2], f32, tag="mv_all")
                    for i in range(nsub):
                        s = b * B + i
                        hd_ps = epsH.tile([P, D_NODE], f32, tag="hd")
                        nc.tensor.matmul(out=hd_ps[:], lhsT=ohT_all[:, s, :],
                                         rhs=hwin[:, 0:D_NODE],
                                         start=True, stop=True)
                        kron = esb.tile([P, D_NODE, D_NODE], bf16, tag="kron")
                        nc.vector.tensor_tensor(
                            out=kron[:],
                            in0=hsel[:, s, 0:D_NODE, None].to_broadcast(
                                [P, D_NODE, D_NODE]),
                            in1=hd_ps[:, None, 0:D_NODE].to_broadcast(
                                [P, D_NODE, D_NODE]),
                            op=mybir.AluOpType.mult)
                        kv = kron[:].rearrange("p a b -> p (a b)")
                        psT = epsT.tile([KCH, NCH, P], bf16, tag="psT")
                        for c in range(NCH):
                            nc.tensor.transpose(
                                out=psT[:, c, :],
                                in_=kv[:, c * KCH:(c + 1) * KCH],
                                identity=ident_bf[:])
                        kron_sb = esb.tile([KCH, NCH, P], bf16, tag="kron_sb")
                        nc.scalar.activation(
                            out=kron_sb[:], in_=psT[:],
                            func=mybir.ActivationFunctionType.Copy)
                        zi = z_all[:, i, :]
                        nmm = NCH + (1 if cfg["has_bk"] else 0)
                        for c in range(NCH):
                            nc.tensor.matmul(out=zi, lhsT=kron_sb[:, c, :],
                                             rhs=wk_sb[:, c, :],
                                             start=(c == 0),
                                             stop=(c == nmm - 1))
                        if cfg["has_bk"]:
                            nc.tensor.matmul(out=zi, lhsT=ones_row[:],
                                             rhs=bk_sb[:], start=False,
                                             stop=True, skip_group_check=True)
                        stats = esmall.tile([P, 6], f32, tag="stats")
                        nc.vector.bn_stats(out=stats[:], in_=zi)
                        nc.vector.bn_aggr(out=mv_all[:, i, :], in_=stats[:])

                    # batched LN scalars over the B subtiles
                    sd_b = esmall.tile([P, B], f32, tag="sd_b")
                    nc.scalar.activation(
                        out=sd_b[:, 0:nsub], in_=mv_all[:, 0:nsub, 1],
                        func=mybir.ActivationFunctionType.Sqrt,
                        bias=eps_t2[:], scale=1.0)
                    rstd_b = esmall.tile([P, B], f32, tag="rstd_b")
                    nc.vector.reciprocal(out=rstd_b[:, 0:nsub],
                                         in_=sd_b[:, 0:nsub])
                    nmr_b = esmall.tile([P, B], f32, tag="nmr_b")
                    nc.vector.scalar_tensor_tensor(
                        out=nmr_b[:, 0:nsub], in0=mv_all[:, 0:nsub, 0],
                        scalar=-1.0, in1=rstd_b[:, 0:nsub],
                        op0=mybir.AluOpType.mult, op1=mybir.AluOpType.mult)

                    for i in range(nsub):
                        s = b * B + i
                        y_sb = esb.tile([P, OUT], bf16, tag="y")
                        func = (mybir.ActivationFunctionType.Relu if simple_k
                                else mybir.ActivationFunctionType.Identity)
                        nc.scalar.activation(out=y_sb[:], in_=z_all[:, i, :],
                                             func=func,
                                             bias=nmr_b[:, i:i + 1],
                                             scale=rstd_b[:, i:i + 1])
                        if not simple_k:
                            if cfg["has_gk"]:
                                nc.vector.tensor_mul(out=y_sb[:], in0=y_sb[:],
                                                     in1=gk_b[:])
                            if cfg["has_betak"]:
                                nc.vector.tensor_add(out=y_sb[:], in0=y_sb[:],
                                                     in1=betak_b[:])
                            nc.vector.tensor_scalar_max(out=y_sb[:],
                                                        in0=y_sb[:],
                                                        scalar1=0.0)

                        nc.tensor.matmul(out=acc_ps[:], lhsT=oh_all[:, s, :],
                                         rhs=y_sb[:], start=(s == 0),
                                         stop=(s == K - 1),
                                         skip_group_check=True)

                out_sb = esb.tile([P, OUT], f32, tag="out_sb")
                nc.vector.tensor_copy(out=out_sb[:], in_=acc_ps[:])
                nc.sync.dma_start(out=out_part[w * P:(w + 1) * P, :],
                                  in_=out_sb[:])

    nc.compile()
    return nc


# --------------------------------------------------------------------------
# entry point
# --------------------------------------------------------------------------
def _install_trace_hook():
    import sys, types, ctypes, contextlib
    if "antenv.axon_hooks" in sys.modules:
        return
    lib = ctypes.CDLL("/opt/axon/libaxon_pjrt.so")
    lib.axon_start_nrt_profile.argtypes = [ctypes.POINTER(ctypes.c_int64),
                                           ctypes.c_size_t]
    lib.axon_start_nrt_profile.restype = ctypes.c_int64
    lib.axon_stop_nrt_profile.argtypes = [ctypes.c_char_p]
    lib.axon_stop_nrt_profile.restype = ctypes.c_int64

    @contextlib.contextmanager
    def _hook(output_dir, device_ids):
        import jax
        jax.devices()
        if device_ids:
            ids = (ctypes.c_int64 * len(device_ids))(*device_ids)
            rc = lib.axon_start_nrt_profile(ids, len(device_ids))
        else:
            rc = lib.axon_start_nrt_profile(None, 0)
        if rc != 0:
            raise RuntimeError(f"axon_start_nrt_profile rc={rc}")
        try:
            yield
        finally:
            n = lib.axon_stop_nrt_profile(str(output_dir).encode())
            print(f"profile: {n} file(s) -> {output_dir}")

    mod = types.ModuleType("antenv.axon_hooks")
    mod.get_axon_ntff_profile_hook = lambda: _hook
    sys.modules["antenv.axon_hooks"] = mod
    from concourse import bass_utils
    bass_utils.upload_artifacts = lambda tmpdir: "local://skipped"


def kernel(**inputs):
    cfg, in_maps = _prep(**inputs)
    key = (cfg["N"], cfg["GF"], cfg["OUT"], cfg["K"], cfg["e_pc"],
           cfg["has_bn"], cfg["has_gn"], cfg["has_betan"], cfg["has_bk"],
           cfg["has_gk"], cfg["has_betak"])
    if key not in _BUILD_CACHE:
        _BUILD_CACHE[key] = _build(cfg)
    nc = _BUILD_CACHE[key]

    if USE_SIM:
        from concourse import bass_interp
        sim = bass_interp.MultiCoreSim(nc, N_CORES)
        for c in range(N_CORES):
            for name, arr in in_maps[c].items():
                sim.cores[c].tensor(name)[:] = arr
        sim.simulate()
        parts = [np.array(sim.cores[c].tensor("out_part"))
                 for c in range(N_CORES)]
        exec_ns = None
    else:
        kw = {}
        if TRACE:
            _install_trace_hook()
            kw = dict(trace=True, tmpdir=TRACE_DIR)
        res = run_bass_kernel_spmd(nc, in_maps, list(range(N_CORES)), **kw)
        parts = [res.results[c]["out_part"] for c in range(N_CORES)]
        exec_ns = res.exec_time_ns
        kernel.last_exec_ns = exec_ns

    out = np.concatenate(parts, axis=0)[:cfg["N"]]
    return out.astype(np.float32)


kernel.last_exec_ns = None
